# revision 1
# baseline (speedup 1.0000x reference)
"""Trainium2 Bass kernel for nn_CombinedConsecutiveAdjustment (B=8192, S=4096).

Math reduction of the reference
-------------------------------
With g in {0,1}:
  - eye_streaks = cumsum(g)*g, so max(eye_streaks) is just the total count of
    ones N1, and argmax is the index of the LAST one, pos (0 if the row is all
    zeros, matching jnp.argmax of an all-zero vector).
  - att_streaks' max is the run of zeros after pos: T = S-1-pos when N1>0.
    (Reference yields -inf when pos==S-1; T=0 fails the >=40 gate identically,
    and the N1==0 case is gated out by N1>=40.)
  - adjustment = (N1>=EYE_TH and T>=ATT_TH) ? MAX_ADJ*(1-exp(-(T-ATT_TH)*3/SAT)) : 0
    where the T>=ATT_TH gate folds into excess = max(T-ATT_TH, 0) because
    excess==0 -> adjustment==0 exactly.
  - out = clip(d*(1-adjustment), MIN_OUT, MAX_OUT)
So per row only two reductions are needed: N1 = sum(g) and pos = max_j(j*g[j]).

Distribution (pure data parallel, per the sharding hint)
--------------------------------------------------------
1024 rows per core on 8 cores. Per core, row r -> (partition p=r//8,
column t=r%8); each (t, chunk) unit is a [128, chunk] slab whose partition
lines are contiguous chunk*4-byte DRAM reads (DMA-friendly).

Per unit pipeline (all engines under the ~47us/core HBM roofline):
  DMA  int32 slab -> SBUF
  ACT  activation(Copy): int32 -> int16 cast, f32 accum_out = chunk popcount
  DVE  tensor_tensor(mult) int16 against a global-valued iota (2x DVE mode)
       tensor_scalar(op1=max, f32 accum_out) = chunk max(j*g[j]) (4x DVE mode)
Chunk accumulators merge in a tiny [128, 8] epilogue computing the scalar
formula and the clipped output, which DMAs back as 8 f32 per partition.
"""

import numpy as np

B = 8192
S = 4096
N_CORES = 8
BC = B // N_CORES          # rows per core = 1024
TPC = BC // 128            # column tiles per core = 8
CHUNK = 1024

EYE_TH = 40.0
ATT_TH = 40.0
MAX_ADJ = 0.05
SAT = 160.0
MIN_OUT = 0.01
MAX_OUT = 1.0

_CACHE = {}


def _build(s=S, tiles=TPC, chunk=CHUNK, gbufs=8, pbufs=4, iota_splits=2):
    import concourse.bacc as bacc
    import concourse.tile as tile
    import concourse.mybir as mybir

    assert s % chunk == 0
    K = s // chunk
    nc = bacc.Bacc(
        "TRN2",
        target_bir_lowering=False,
        debug=False,
        num_devices=N_CORES,
    )
    f32 = mybir.dt.float32
    i32 = mybir.dt.int32
    i16 = mybir.dt.int16
    bc = 128 * tiles

    g_dram = nc.dram_tensor("g", [bc, s], i32, kind="ExternalInput").ap()
    d_dram = nc.dram_tensor("d", [bc, 1], f32, kind="ExternalInput").ap()
    o_dram = nc.dram_tensor("o", [bc, 1], f32, kind="ExternalOutput").ap()

    g_view = g_dram.rearrange("(p t) s -> t p s", t=tiles)    # [t][128, s]
    d_view = d_dram.rearrange("(p t) o -> p (t o)", t=tiles)  # [128, tiles]
    o_view = o_dram.rearrange("(p t) o -> p (t o)", t=tiles)  # [128, tiles]

    Copy = mybir.ActivationFunctionType.Copy
    Exp = mybir.ActivationFunctionType.Exp
    A = mybir.AluOpType

    with tile.TileContext(nc) as tc:
        with (
            tc.tile_pool(name="gpool", bufs=gbufs) as gpool,
            tc.tile_pool(name="fpool", bufs=pbufs) as fpool,
            tc.tile_pool(name="ppool", bufs=pbufs) as ppool,
            tc.tile_pool(name="small", bufs=1) as small,
        ):
            # iota carrying global column values, emitted in pieces so the
            # first chunk's compute starts before the whole iota exists
            iota = small.tile([128, s], i16)
            isplit = max(iota_splits, 1)
            istep = s // isplit
            for k in range(isplit):
                nc.gpsimd.iota(iota[:, k * istep : (k + 1) * istep],
                               pattern=[[1, istep]], base=k * istep,
                               channel_multiplier=0)

            pos_acc = small.tile([128, tiles * K], f32)
            cnt_acc = small.tile([128, tiles * K], f32)
            d_sb = small.tile([128, tiles], f32)

            for t in range(tiles):
                for k in range(K):
                    col = t * K + k
                    lo, hi = k * chunk, (k + 1) * chunk
                    gt = gpool.tile([128, chunk], i32, name="gt")
                    nc.sync.dma_start(out=gt[:], in_=g_view[t][:, lo:hi])
                    gf = fpool.tile([128, chunk], i16, name="gf")
                    nc.scalar.activation(out=gf[:], in_=gt[:], func=Copy,
                                         accum_out=cnt_acc[:, col : col + 1])
                    prod = ppool.tile([128, chunk], i16, name="prod")
                    nc.vector.tensor_tensor(out=prod[:], in0=gf[:],
                                            in1=iota[:, lo:hi], op=A.mult)
                    nc.vector.tensor_scalar(out=prod[:], in0=prod[:],
                                            scalar1=0, scalar2=None,
                                            op0=A.max, op1=A.max,
                                            accum_out=pos_acc[:, col : col + 1])

            # drowsiness load is tiny; emitted late so it never delays the
            # first gesture chunk on the DMA engines
            nc.sync.dma_start(out=d_sb[:], in_=d_view)

            # ---- epilogue on [128, tiles] ----
            pos_f = small.tile([128, tiles], f32)
            cnt_f = small.tile([128, tiles], f32)
            if K > 1:
                pos_v = pos_acc[:].rearrange("p (t k) -> p t k", k=K)
                cnt_v = cnt_acc[:].rearrange("p (t k) -> p t k", k=K)
                nc.vector.tensor_reduce(pos_f[:], pos_v,
                                        axis=mybir.AxisListType.X, op=A.max)
                nc.vector.tensor_reduce(cnt_f[:], cnt_v,
                                        axis=mybir.AxisListType.X, op=A.add)
            else:
                pos_f = pos_acc
                cnt_f = cnt_acc

            t_f = small.tile([128, tiles], f32)
            excess = small.tile([128, tiles], f32)
            e_f = small.tile([128, tiles], f32)
            adjraw = small.tile([128, tiles], f32)
            conda = small.tile([128, tiles], f32)
            adj = small.tile([128, tiles], f32)
            da = small.tile([128, tiles], f32)
            outp = small.tile([128, tiles], f32)
            res = small.tile([128, tiles], f32)

            # T = (S-1) - pos
            nc.vector.tensor_scalar(out=t_f[:], in0=pos_f[:],
                                    scalar1=-1.0, scalar2=float(s - 1),
                                    op0=A.mult, op1=A.add)
            # excess = max(T - ATT_TH, 0)
            nc.vector.tensor_scalar(out=excess[:], in0=t_f[:],
                                    scalar1=ATT_TH, scalar2=0.0,
                                    op0=A.subtract, op1=A.max)
            # e = exp(-excess * 3/SAT)
            nc.scalar.activation(out=e_f[:], in_=excess[:], func=Exp,
                                 scale=-3.0 / SAT)
            # adjraw = MAX_ADJ * (1 - e)
            nc.vector.tensor_scalar(out=adjraw[:], in0=e_f[:],
                                    scalar1=-MAX_ADJ, scalar2=MAX_ADJ,
                                    op0=A.mult, op1=A.add)
            # conda = (cnt >= EYE_TH)
            nc.vector.tensor_scalar(out=conda[:], in0=cnt_f[:],
                                    scalar1=EYE_TH, scalar2=None, op0=A.is_ge)
            nc.vector.tensor_tensor(out=adj[:], in0=adjraw[:], in1=conda[:], op=A.mult)
            nc.vector.tensor_tensor(out=da[:], in0=d_sb[:], in1=adj[:], op=A.mult)
            nc.vector.tensor_tensor(out=outp[:], in0=d_sb[:], in1=da[:], op=A.subtract)
            nc.vector.tensor_scalar(out=res[:], in0=outp[:],
                                    scalar1=MIN_OUT, scalar2=MAX_OUT,
                                    op0=A.max, op1=A.min)
            nc.sync.dma_start(out=o_view, in_=res[:])

    nc.compile()
    return nc


def _get_nc(**kw):
    key = tuple(sorted(kw.items()))
    if key not in _CACHE:
        _CACHE[key] = _build(**kw)
    return _CACHE[key]


def kernel(drowsiness_index, gesture_sequence):
    from concourse.bass_utils import run_bass_kernel_spmd

    d = np.asarray(drowsiness_index, dtype=np.float32).reshape(B, 1)
    g = np.ascontiguousarray(np.asarray(gesture_sequence, dtype=np.int32).reshape(B, S))

    nc = _get_nc()
    in_maps = [
        {"g": g[c * BC : (c + 1) * BC], "d": d[c * BC : (c + 1) * BC]}
        for c in range(N_CORES)
    ]
    r = run_bass_kernel_spmd(nc, in_maps, list(range(N_CORES)))
    out = np.concatenate([r.results[c]["o"] for c in range(N_CORES)], axis=0)
    return out.reshape(B, 1).astype(np.float32, copy=False)



# revision 8
# speedup vs baseline: 1.0095x; 1.0095x over previous
"""Trainium2 Bass kernel for nn_CombinedConsecutiveAdjustment (B=8192, S=4096).

Math reduction of the reference
-------------------------------
With g in {0,1}:
  - max(eye_streaks) is the total count of ones N1; argmax is the index of
    the LAST one, pos (0 for an all-zero row, matching argmax of zeros).
  - The post-pos attention streak is T = S-1-pos; gates fold so that
    adjustment = (N1>=40) ? 0.05*(1-exp(-max(T-40,0)*3/160)) : 0
    and out = clip(d*(1-adjustment), 0.01, 1.0).
Per row only two reductions: N1 = sum(g), pos = max_j(j*g[j]).

Distribution: pure data parallel, 1024 rows/core on 8 cores. Row r ->
(partition p=r//8, column tile t=r%8); slab partition lines are contiguous
DRAM reads (up to 16KB), streaming the 16.8MB/core at the HBM roofline.

Schedule (v5, from TimelineSim trace analysis + a schedule model)
----------------------------------------------------------------
The 46.6us HBM stream is the floor; overheads are hidden around it:
  - Slabs are issued with descending widths obeying act(w_prev) <=
    dma(w_next) so the Activation-engine count pass never falls behind
    the stream; the exposed post-stream tail is one tiny slab's compute.
    Big 16KB-line slabs stream first (tiles 0..4 and the leftovers of
    tiles 5/6 whose small-slab deficit hides in big-slab slack); the
    taper's suffix belongs entirely to tile 7.
  - Per slab: Activation casts int32->int16 (accum_out = ones count),
    DVE multiplies by a global-valued iota, then a tensor_scalar whose
    scalar1 is the tile's PREVIOUS accumulator (AP) max-chains the
    last-one position -- no final cross-slab reduce needed.
  - Count partials fold pairwise on the idle Pool engine as they land,
    leaving a single add after the final slab.
  - Fused epilogue on [128,8]: c=(cnt>=40); excess=max(S-41-pos,0);
    e=exp(-3/160*excess*c); out=clip(d*(0.05e+0.95), .01, 1).
"""

import numpy as np

B = 8192
S = 4096
N_CORES = 8
BC = B // N_CORES          # rows per core = 1024
TPC = BC // 128            # column tiles per core = 8

EYE_TH = 40.0
ATT_TH = 40.0
MAX_ADJ = 0.05
SAT = 160.0
MIN_OUT = 0.01
MAX_OUT = 1.0

# (tile, lo, hi) in exact DMA issue order; widths descend through the taper
_SLABS = [
    (0, 0, 4096),
    (5, 3968, 4096),
    (6, 3968, 4096),
    (1, 0, 4096),
    (2, 0, 4096),
    (3, 0, 4096),
    (4, 0, 4096),
    (5, 0, 2688),
    (6, 0, 2176),
    (6, 2176, 3968),
    (7, 0, 1536),
    (5, 2688, 3968),
    (7, 1536, 2560),
    (7, 2560, 3328),
    (7, 3328, 3712),
    (7, 3712, 4096),
]

_CACHE = {}


def _build(gbufs=6, sbufs=3):
    import concourse.bacc as bacc
    import concourse.tile as tile
    import concourse.mybir as mybir

    nc = bacc.Bacc(
        "TRN2",
        target_bir_lowering=False,
        debug=False,
        num_devices=N_CORES,
    )
    f32 = mybir.dt.float32
    i32 = mybir.dt.int32
    i16 = mybir.dt.int16
    tiles = TPC
    bc = 128 * tiles

    g_dram = nc.dram_tensor("g", [bc, S], i32, kind="ExternalInput").ap()
    d_dram = nc.dram_tensor("d", [bc, 1], f32, kind="ExternalInput").ap()
    o_dram = nc.dram_tensor("o", [bc, 1], f32, kind="ExternalOutput").ap()

    g_view = g_dram.rearrange("(p t) s -> t p s", t=tiles)    # [t][128, s]
    d_view = d_dram.rearrange("(p t) o -> p (t o)", t=tiles)  # [128, tiles]
    o_view = o_dram.rearrange("(p t) o -> p (t o)", t=tiles)  # [128, tiles]

    Copy = mybir.ActivationFunctionType.Copy
    Exp = mybir.ActivationFunctionType.Exp
    A = mybir.AluOpType

    nslabs = {t: sum(1 for s in _SLABS if s[0] == t) for t in range(tiles)}

    with tile.TileContext(nc) as tc:
        with (
            tc.tile_pool(name="gpool", bufs=gbufs) as gpool,
            tc.tile_pool(name="cpool", bufs=sbufs) as cpool,
            tc.tile_pool(name="ppool", bufs=sbufs) as ppool,
            tc.tile_pool(name="small", bufs=1) as small,
        ):
            # iota carrying global column values (i16: j*g fits 4095);
            # emitted in halves so slab 0's DVE op never waits on the whole
            iota = small.tile([128, S], i16)
            for k in range(2):
                nc.gpsimd.iota(iota[:, k * (S // 2): (k + 1) * (S // 2)],
                               pattern=[[1, S // 2]], base=k * (S // 2),
                               channel_multiplier=0)

            pos_f = small.tile([128, tiles], f32)
            cnt_f = small.tile([128, tiles], f32)
            pchain, cparts, cruns = {}, {}, {}
            for t in range(tiles):
                if nslabs[t] > 1:
                    pchain[t] = small.tile([128, nslabs[t] - 1], f32,
                                           name=f"pchain{t}")
                    cparts[t] = small.tile([128, nslabs[t]], f32,
                                           name=f"cparts{t}")
                    cruns[t] = small.tile([128, nslabs[t] - 1], f32,
                                          name=f"cruns{t}")
            d_sb = small.tile([128, tiles], f32)

            seen = {t: 0 for t in range(tiles)}
            for t, lo, hi in _SLABS:
                w = hi - lo
                si, ns = seen[t], nslabs[t]
                seen[t] += 1
                last = si == ns - 1

                gt = gpool.tile([128, w], i32, name="gt")
                nc.sync.dma_start(out=gt[:], in_=g_view[t][:, lo:hi])

                # cast to i16 for 2x DVE throughput; accum_out = ones count
                cnt_dst = (cnt_f[:, t: t + 1] if ns == 1
                           else cparts[t][:, si: si + 1])
                gf = cpool.tile([128, w], i16, name="gf")
                nc.scalar.activation(out=gf[:], in_=gt[:], func=Copy,
                                     accum_out=cnt_dst)

                # last-one position: prod = g*iota, then max-chain through
                # the tile's previous accumulator (scalar1 as AP)
                prod = ppool.tile([128, w], i16, name="prod")
                nc.vector.tensor_tensor(out=prod[:], in0=gf[:],
                                        in1=iota[:, lo:hi], op=A.mult)
                init = 0.0 if si == 0 else pchain[t][:, si - 1: si]
                pos_dst = (pos_f[:, t: t + 1] if last
                           else pchain[t][:, si: si + 1])
                nc.vector.tensor_scalar(out=prod[:], in0=prod[:],
                                        scalar1=init, scalar2=None,
                                        op0=A.max, op1=A.max,
                                        accum_out=pos_dst)

                # fold count partials on Pool as they land; the final fold
                # writes cnt_f (only tile 7's last fold is exposed)
                if ns > 1 and si >= 1:
                    prev = (cparts[t][:, 0:1] if si == 1
                            else cruns[t][:, si - 2: si - 1])
                    dst = (cnt_f[:, t: t + 1] if last
                           else cruns[t][:, si - 1: si])
                    nc.gpsimd.tensor_tensor(out=dst, in0=prev,
                                            in1=cparts[t][:, si: si + 1],
                                            op=A.add)

            # drowsiness load: tiny, after the gesture stream
            nc.sync.dma_start(out=d_sb[:], in_=d_view)

            # ---- fused epilogue on [128, tiles] ----
            cgate = small.tile([128, tiles], f32)
            t1 = small.tile([128, tiles], f32)
            excess = small.tile([128, tiles], f32)
            eg = small.tile([128, tiles], f32)
            e_f = small.tile([128, tiles], f32)
            m_f = small.tile([128, tiles], f32)
            outp = small.tile([128, tiles], f32)
            res = small.tile([128, tiles], f32)

            # c = (cnt >= EYE_TH) on Pool, parallel with the DVE pos path
            nc.gpsimd.tensor_scalar(out=cgate[:], in0=cnt_f[:],
                                    scalar1=EYE_TH, scalar2=None, op0=A.is_ge)
            nc.vector.tensor_scalar(out=t1[:], in0=pos_f[:],
                                    scalar1=-1.0, scalar2=float(S - 1 - ATT_TH),
                                    op0=A.mult, op1=A.add)
            nc.vector.tensor_scalar(out=excess[:], in0=t1[:],
                                    scalar1=0.0, scalar2=None, op0=A.max)
            nc.vector.tensor_tensor(out=eg[:], in0=excess[:], in1=cgate[:],
                                    op=A.mult)
            nc.scalar.activation(out=e_f[:], in_=eg[:], func=Exp,
                                 scale=-3.0 / SAT)
            nc.vector.tensor_scalar(out=m_f[:], in0=e_f[:],
                                    scalar1=MAX_ADJ, scalar2=1.0 - MAX_ADJ,
                                    op0=A.mult, op1=A.add)
            nc.vector.tensor_tensor(out=outp[:], in0=d_sb[:], in1=m_f[:],
                                    op=A.mult)
            nc.vector.tensor_scalar(out=res[:], in0=outp[:],
                                    scalar1=MIN_OUT, scalar2=MAX_OUT,
                                    op0=A.max, op1=A.min)
            nc.sync.dma_start(out=o_view, in_=res[:])

    nc.compile()
    return nc


def _get_nc(**kw):
    key = tuple(sorted(kw.items()))
    if key not in _CACHE:
        _CACHE[key] = _build(**kw)
    return _CACHE[key]


def kernel(drowsiness_index, gesture_sequence):
    from concourse.bass_utils import run_bass_kernel_spmd

    d = np.asarray(drowsiness_index, dtype=np.float32).reshape(B, 1)
    g = np.ascontiguousarray(np.asarray(gesture_sequence, dtype=np.int32).reshape(B, S))

    nc = _get_nc()
    in_maps = [
        {"g": g[c * BC: (c + 1) * BC], "d": d[c * BC: (c + 1) * BC]}
        for c in range(N_CORES)
    ]
    r = run_bass_kernel_spmd(nc, in_maps, list(range(N_CORES)))
    out = np.concatenate([r.results[c]["o"] for c in range(N_CORES)], axis=0)
    return out.reshape(B, 1).astype(np.float32, copy=False)
